# revision 43
# baseline (speedup 1.0000x reference)
"""Adaptive-softmax CE loss on 8 TRN2 NeuronCores.

Strategy: the CE is masked per cluster, so tail logsumexps are only
needed for tokens IN that cluster (~16% for tail0, ~80% for tail1).
  - Head (2002-wide lse, all 4096 tokens): data-parallel, 512 tokens/core.
  - Tails: host compacts cluster tokens (T0~633 -> 640, T1~3293 -> 3328),
    then TENSOR-PARALLEL vocab split: every core computes logits for ALL
    compacted tail tokens but only its 1/8 vocab slice (1000 of 8000,
    5000 of 40000).  Host sums the 8 per-core sum-exp partials per token
    (sharded logsumexp) - no collectives.
This cuts ScalarE exp work from 25.6M to ~18.4M elems/core (the hard
floor: exp runs only on ScalarE at 128 lanes * 1.2 GHz) and makes all
weights SBUF-resident.

The tail hidden states h = fp8((x8 @ 64*w1)/64) and the label-logit
dots are computed ON THE HOST (cheap: ~2.4 GFLOP numpy); the device
consumes the SAME fp8 h for the lse logits, so lse - label_logit
cancels fp8 noise exactly, and the device runs a single stream of
logit matmuls + exp with no phase-A, no gathers.  Total device input:
~6.5MB/core, all partition-major so every DMA is ~128-256 contiguous
descriptors (descriptor GENERATION was the startup bottleneck).

Per-tile reduction: ACT accum_out for t1 tile0; DVE tensor_reduce over
bf16 exp tiles for the rest.  Head/t0 tiles interleave between t1
tiles and real-shaped filler matmuls pad spare PSUM columns: the PE
HAM clock gate re-throttles the tensor engine to 1.2GHz when its
activity-window utilization drops, which would let ACT starve.

Numerics: fp8 DoubleRow matmuls with x64-scaled weights, undone for
free via exp(x/64); host finishes in float64 (log, masks, average).
"""

import numpy as np
import ml_dtypes

CUTOFF = [2000, 10000, 50000]
N_TOK = 4096
D = 1024
N_CORES = 8
TOK_PER_CORE = N_TOK // N_CORES          # 512
N_BLK = TOK_PER_CORE // 128              # 4 head token blocks
K0 = 8                                   # 1024/128
K1 = 2                                   # 256/128
N_HEAD = CUTOFF[0] + 2                   # 2002
V0 = CUTOFF[1] - CUTOFF[0]               # 8000
V1 = CUTOFF[2] - CUTOFF[1]               # 40000
V0S = V0 // N_CORES                      # 1000 per-core tail0 vocab slice
V1S = V1 // N_CORES                      # 5000 per-core tail1 vocab slice
WSCALE = 64.0
T1T = [2048, 1536, 1416]                 # t1 per-block vocab tile widths

BF16 = ml_dtypes.bfloat16
FP8 = ml_dtypes.float8_e4m3

_cache = {}


def _subs(width, step=512):
    out, o = [], 0
    while o < width:
        out.append((o, min(step, width - o)))
        o += min(step, width - o)
    return out


def _build_nc(b0, b1, use_bias):
    import concourse.bass as bass
    import concourse.bacc as bacc
    import concourse.mybir as mybir
    from concourse import tile

    t0c = b0 * 128
    t1c = b1 * 128
    nhh = (N_HEAD + 1023) // 1024        # head halves

    dt = mybir.dt
    nc = bacc.Bacc(None)

    EXP = mybir.ActivationFunctionType.Exp
    ADD = mybir.AluOpType.add
    DR = mybir.MatmulPerfMode.DoubleRow
    X = mybir.AxisListType.X
    PSUM = bass.MemorySpace.PSUM

    pre_p = nc.declare_dram_parameter("pre", [128, 16], dt.float8e4, isOutput=False)
    xh8_p = nc.declare_dram_parameter("xh8", [128, K0, TOK_PER_CORE], dt.float8e4, isOutput=False)
    hwt8_p = nc.declare_dram_parameter("hwt8", [128, nhh * 2, K0, 512], dt.float8e4, isOutput=False)
    if use_bias:
        hbias_p = nc.declare_dram_parameter("hbias", [1, N_HEAD], dt.bfloat16, isOutput=False)
    ht0_p = nc.declare_dram_parameter("ht0", [128, K0, t0c], dt.float8e4, isOutput=False)
    ht1_p = nc.declare_dram_parameter("ht1", [128, K1, t1c], dt.float8e4, isOutput=False)
    w2t0_p = nc.declare_dram_parameter("w2t0", [128, K0, V0S], dt.float8e4, isOutput=False)
    w2t1_p = nc.declare_dram_parameter("w2t1", [128, 3, K1, 2048], dt.float8e4, isOutput=False)

    ncols = (2 * N_BLK + 1) + b0 + 3 * b1
    out_s_p = nc.declare_dram_parameter("out_s", [128, ncols], dt.float32, isOutput=True)

    cols = []

    with tile.TileContext(nc) as tc:
        with (
            tc.tile_pool(name="res", bufs=1) as res,
            tc.tile_pool(name="es", bufs=12) as es,
        ):
            pre = res.tile([128, 16], dt.float8e4, tag="pre")
            xh8 = res.tile([128, K0, TOK_PER_CORE], dt.float8e4, tag="xh8")
            hwt8 = res.tile([128, nhh * 2, K0, 512], dt.float8e4, tag="hwt8")
            if use_bias:
                hbias = res.tile([1, N_HEAD], dt.bfloat16, tag="hbias")
            ht0_8 = res.tile([128, K0, t0c], dt.float8e4, tag="ht0_8")
            ht1_8 = res.tile([128, K1, t1c], dt.float8e4, tag="ht1_8")
            w2t0 = res.tile([128, K0, V0S], dt.float8e4, tag="w2t0")
            w2t1 = res.tile([128, 3, K1, 2048], dt.float8e4, tag="w2t1")
            sall = res.tile([128, ncols], dt.float32, tag="sall")
            ones = res.tile([128, 1], dt.bfloat16, tag="ones")
            ones1 = res.tile([1, 128], dt.bfloat16, tag="ones1")

            nc.gpsimd.memset(ones[:], 1.0)
            nc.gpsimd.memset(ones1[:], 1.0)

            with tc.tile_pool(name="pc", bufs=2, space=PSUM) as pcp:

                def exp_reduce(pc, w, mode, kind, b):
                    col = len(cols)
                    cols.append((kind, b))
                    if mode == "acc":
                        nc.scalar.activation(
                            pc[:, :w], pc[:, :w], EXP,
                            scale=1.0 / WSCALE, accum_out=sall[:, col:col + 1],
                        )
                    else:
                        et = es.tile([128, 2048], dt.bfloat16, tag="e")
                        nc.scalar.activation(et[:, :w], pc[:, :w], EXP,
                                             scale=1.0 / WSCALE)
                        nc.vector.tensor_reduce(sall[:, col:col + 1], et[:, :w],
                                                axis=X, op=ADD)

                def mm_group(pc, sl, btok, kk, lhs3, rhs3, bias=False, rbase=0,
                             bbase=0):
                    rsl = slice(rbase + sl.start, rbase + sl.stop)
                    for c in range(kk // 2):
                        nc.tensor.matmul(
                            pc[:, sl],
                            lhsT=lhs3[:, 2 * c:2 * c + 2, btok * 128:(btok + 1) * 128],
                            rhs=rhs3[:, 2 * c:2 * c + 2, rsl],
                            start=(c == 0),
                            stop=(c == kk // 2 - 1 and not bias),
                            perf_mode=DR,
                        )
                    if bias:
                        bsl = slice(bbase + sl.start, bbase + sl.stop)
                        nc.tensor.matmul(pc[:, sl], lhsT=ones1[:],
                                         rhs=hbias[0:1, bsl], start=False, stop=True)

                def pe_filler(pc, b, pofs=1536):
                    # real-shaped dummy matmul into unused PSUM columns of a
                    # narrow tile: PE-HAM keep-warm work; subtile deps keep
                    # the tile's ACT read independent of this write
                    bb = (b % b1) * 128
                    nc.tensor.matmul(
                        pc[:, pofs:pofs + 512],
                        lhsT=ht1_8[:, 0:2, bb:bb + 128],
                        rhs=w2t1[:, 0, 0:2, 0:512],
                        start=True, stop=True, perf_mode=DR,
                    )

                def emit_head(b, hf, split=False):
                    width = min(1024, N_HEAD - hf * 1024)
                    pc = pcp.tile([128, 2048], dt.float32, tag="pc")
                    for qi, (off, w) in enumerate(_subs(width)):
                        mm_group(pc, slice(off, off + w), b, K0, xh8,
                                 hwt8[:, hf * 2 + qi], bias=use_bias,
                                 rbase=-off, bbase=hf * 1024)
                        if split:
                            exp_reduce(pc[:, off:], w, "dve", "h", b)
                    if not split:
                        exp_reduce(pc, width, "dve", "h", b)

                def emit_t0(b):
                    pc = pcp.tile([128, 2048], dt.float32, tag="pc")
                    for off, w in _subs(V0S):
                        mm_group(pc, slice(off, off + w), b, K0, ht0_8, w2t0)
                    pe_filler(pc, b, 1024)
                    exp_reduce(pc, V0S, "dve", "t0", b)

                def emit_t1(b, j, j0mode="acc"):
                    width = T1T[j] if j < 2 else V1S - T1T[0] - T1T[1]
                    pc = pcp.tile([128, 2048], dt.float32, tag="pc")
                    for off, w in _subs(width):
                        mm_group(pc, slice(off, off + w), b, K1, ht1_8,
                                 w2t1[:, j])
                    if j >= 1:
                        pe_filler(pc, b + j, width)
                    mode = j0mode if j == 0 else (
                        "acc" if b >= b1 - 1 else "dve")
                    exp_reduce(pc, width, mode, "t1", b)

                # startup: tiny prewarm absorbs DMA spin-up; head inputs
                # first, then the t1 stream inputs; dummy matmuls warm the
                # PE HAM clock gate while the first DMAs land
                h1h = (t1c // 2) // 128 * 128
                nc.sync.dma_start(pre[:], pre_p[:])
                nc.sync.dma_start(xh8[:], xh8_p[:])
                nc.gpsimd.dma_start(hwt8[:, 0], hwt8_p[:, 0])
                if use_bias:
                    nc.sync.dma_start(hbias[:], hbias_p[:])
                nc.gpsimd.dma_start(hwt8[:, 1], hwt8_p[:, 1])
                nc.sync.dma_start(ht1_8[:, :, 0:h1h], ht1_p[:, :, 0:h1h])
                nc.sync.dma_start(w2t1[:, 0], w2t1_p[:, 0])
                pw = pcp.tile([128, 2048], dt.float32, tag="pc")
                for i in range(26):
                    # warmup keyed on the tiny 'pre' DMA (~9.6us): holds PE
                    # busy until the first real inputs land so HAM is warm
                    # when the real matmuls start
                    nc.tensor.matmul(pw[0:8, 0:8], lhsT=pre[:, 0:8],
                                     rhs=pre[:, 0:8], start=(i == 0),
                                     stop=(i == 25))
                emit_head(0, 0, split=True)
                nc.sync.dma_start(ht1_8[:, :, h1h:t1c], ht1_p[:, :, h1h:t1c])
                nc.sync.dma_start(w2t1[:, 1], w2t1_p[:, 1])
                emit_head(1, 0)
                nc.gpsimd.dma_start(hwt8[:, 2], hwt8_p[:, 2])
                nc.gpsimd.dma_start(hwt8[:, 3], hwt8_p[:, 3])
                nc.sync.dma_start(w2t1[:, 2], w2t1_p[:, 2])
                emit_head(2, 0)
                emit_head(1, 1)

                # spread head/t0 tiles (ACT-productive, PE-heavy) evenly
                # between the t1 blocks
                extras = [[] for _ in range(b1)]

                def put(bi, item):
                    extras[min(max(bi, 0), b1 - 1)].append(item)

                put(0, ("dma", "ht0"))
                put(1, ("dma", "w2t0"))
                hds = [(0, 1), (2, 1), (3, 0), (3, 1)]
                units = []
                for i in range(max(b0, len(hds))):
                    if i < len(hds):
                        units.append(("hd",) + hds[i])
                    if i < b0:
                        units.append(("t0", i))
                mix = list(range(2, b1))
                step = max(1.0, len(mix) / max(1, len(units)))
                for i, unit in enumerate(units):
                    put(mix[min(int(i * step), len(mix) - 1)], unit)

                def run_extra(e):
                    if e[0] == "hd":
                        emit_head(e[1], e[2])
                    elif e[0] == "t0":
                        emit_t0(e[1])
                    elif e[0] == "dma":
                        if e[1] == "ht0":
                            nc.sync.dma_start(ht0_8[:], ht0_p[:])
                        elif e[1] == "w2t0":
                            nc.sync.dma_start(w2t0[:], w2t0_p[:])

                flush_pts = sorted({max(0, b1 // 2), max(0, b1 - 1 - 1)})
                nflush = 0
                for b in range(b1):
                    u = extras[b]
                    heavy = any(e[0] in ("hd", "t0") for e in u)
                    j0mode = "dve" if (not heavy and b < b1 - 2) else "acc"
                    for j in range(3):
                        if j < len(u):
                            run_extra(u[j])
                        emit_t1(b, j, j0mode)
                    for e in u[3:]:
                        run_extra(e)
                    if b in flush_pts:
                        nc.sync.dma_start(out_s_p[:, nflush:len(cols)],
                                          sall[:, nflush:len(cols)])
                        nflush = len(cols)

            nc.scalar.dma_start(out_s_p[:, nflush:ncols], sall[:, nflush:ncols])

    nc.compile()
    return nc, cols


def _prep_inputs(w_in, target, head_w, head_b, tail0_w1, tail0_w2, tail1_w1, tail1_w2):
    f32 = np.float32
    w_in = np.asarray(w_in, f32)
    target = np.asarray(target).astype(np.int64)
    head_w = np.asarray(head_w, f32)
    head_b = np.asarray(head_b, f32)
    t0w1 = np.asarray(tail0_w1, f32)
    t0w2 = np.asarray(tail0_w2, f32)
    t1w1 = np.asarray(tail1_w1, f32)
    t1w2 = np.asarray(tail1_w2, f32)

    c0, c1, c2 = CUTOFF
    mask0 = (target >= c0) & (target < c1)
    mask1 = (target >= c1) & (target < c2)
    idx0 = np.where(mask0)[0]
    idx1 = np.where(mask1)[0]
    t0n, t1n = len(idx0), len(idx1)
    b0 = max(1, -(-t0n // 128))
    b1 = max(1, -(-t1n // 128))
    t0c, t1c = b0 * 128, b1 * 128
    nhh = (N_HEAD + 1023) // 1024
    first_t = np.where(mask0, c0, np.where(mask1, c0 + 1, target))
    use_bias = bool(np.any(head_b))

    def pmajor(a, k):
        # [k*128, F] -> [128, k, F] partition-major contiguous
        return np.ascontiguousarray(
            a.reshape(k, 128, a.shape[1]).transpose(1, 0, 2))

    x8_all = w_in.T.astype(FP8)                    # [1024, N_TOK]
    w1t0_8 = (t0w1.T * WSCALE).astype(FP8)         # [1024, 1024]
    w1t1_8 = (t1w1.T * WSCALE).astype(FP8)         # [1024, 256]
    hw_8 = (head_w.T * WSCALE).astype(FP8)         # [1024, 2002]
    w2t0_8 = (t0w2.T * WSCALE).astype(FP8)         # [1024, 8000]
    w2t1_8 = (t1w2.T * WSCALE).astype(FP8)         # [256, 40000]
    hbias = (head_b[None, :] * WSCALE).astype(BF16)

    # head weights, quarter-major [128, nhh*2, K0, 512]
    hw_pad = np.zeros((1024, nhh * 1024), FP8)
    hw_pad[:, :N_HEAD] = hw_8
    hwt8 = np.ascontiguousarray(
        hw_pad.reshape(K0, 128, nhh * 2, 512).transpose(1, 2, 0, 3))

    # ---- host-side tail hidden states + label-logit dots (the device
    # consumes the SAME fp8 h, so lse - dot cancels fp8 noise) ----
    f = np.float32
    h0 = ((x8_all[:, idx0].astype(f).T @ w1t0_8.astype(f)) / WSCALE).astype(FP8)
    h1 = ((x8_all[:, idx1].astype(f).T @ w1t1_8.astype(f)) / WSCALE).astype(FP8)
    ll0 = np.einsum("tf,ft->t", h0.astype(f),
                    w2t0_8[:, target[idx0] - c0].astype(f)) / WSCALE
    ll1 = np.einsum("tf,ft->t", h1.astype(f),
                    w2t1_8[:, target[idx1] - c1].astype(f)) / WSCALE
    llh = (np.einsum("ft,ft->t", x8_all.astype(f),
                     hw_8.astype(f)[:, first_t]) / WSCALE + head_b[first_t])

    def padT8(a, tcap):  # fp8 [T, F] -> fp8 [F, tcap]
        out = np.zeros((a.shape[1], tcap), FP8)
        out[:, :a.shape[0]] = a.T
        return out

    ht0 = pmajor(padT8(h0, t0c), K0)               # [128, K0, t0c]
    ht1 = pmajor(padT8(h1, t1c), K1)               # [128, K1, t1c]

    pre = np.zeros((128, 16), FP8)
    in_maps = []
    for c in range(N_CORES):
        sl = slice(c * TOK_PER_CORE, (c + 1) * TOK_PER_CORE)
        # per-core tail1 vocab slice, tile-major [128, 3, K1, 2048]
        w2t1s = np.zeros((256, 3, 2048), FP8)
        base = c * V1S
        o = 0
        for j, wdt in enumerate(T1T):
            w2t1s[:, j, :wdt] = w2t1_8[:, base + o:base + o + wdt]
            o += wdt
        w2t1m = np.ascontiguousarray(
            w2t1s.reshape(K1, 128, 3, 2048).transpose(1, 2, 0, 3))
        m = {
            "pre": pre,
            "xh8": pmajor(x8_all[:, sl], K0),
            "hwt8": hwt8,
            "ht0": ht0, "ht1": ht1,
            "w2t0": pmajor(w2t0_8[:, c * V0S:(c + 1) * V0S], K0),
            "w2t1": w2t1m,
        }
        if use_bias:
            m["hbias"] = hbias
        in_maps.append(m)
    meta = (b0, b1, t0n, t1n, use_bias, llh, ll0, ll1)
    return in_maps, meta


def _combine(results, cols, meta):
    b0, b1, t0n, t1n, use_bias, llh, ll0, ll1 = meta
    S0 = np.zeros((128, b0))
    S1 = np.zeros((128, b1))
    logSh = np.zeros(N_TOK)
    for c in range(N_CORES):
        S = results[c]["out_s"].astype(np.float64)
        Sh = np.zeros((128, N_BLK))
        for j, (k, b) in enumerate(cols):
            if k == "h":
                Sh[:, b] += S[:, j]
            elif k == "t0":
                S0[:, b] += S[:, j]
            else:
                S1[:, b] += S[:, j]
        # token (p, b) -> global index c*512 + b*128 + p
        logSh[c * TOK_PER_CORE:(c + 1) * TOK_PER_CORE] = np.log(Sh).T.reshape(-1)
    total = (logSh - llh).sum()
    total += (np.log(S0.T.reshape(-1)[:t0n]) - ll0).sum()
    total += (np.log(S1.T.reshape(-1)[:t1n]) - ll1).sum()
    return np.float32(total / N_TOK)


def _run(inputs, trace=False):
    from concourse.bass_utils import run_bass_kernel_spmd

    in_maps, meta = _prep_inputs(**inputs)
    key = (meta[0], meta[1], meta[4])
    if key not in _cache:
        _cache[key] = _build_nc(*key)
    nc, cols = _cache[key]
    res = run_bass_kernel_spmd(nc, in_maps, core_ids=list(range(N_CORES)), trace=trace)
    loss = _combine(res.results, cols, meta)
    return loss, res


def kernel(**inputs) -> np.ndarray:
    loss, _ = _run(inputs, trace=False)
    return loss


# revision 44
# speedup vs baseline: 1.0065x; 1.0065x over previous
"""Adaptive-softmax CE loss on 8 TRN2 NeuronCores.

Strategy: the CE is masked per cluster, so tail logsumexps are only
needed for tokens IN that cluster (~16% for tail0, ~80% for tail1).
  - Head (2002-wide lse, all 4096 tokens): data-parallel, 512 tokens/core.
  - Tails: host compacts cluster tokens (T0~633 -> 640, T1~3293 -> 3328),
    then TENSOR-PARALLEL vocab split: every core computes logits for ALL
    compacted tail tokens but only its 1/8 vocab slice (1000 of 8000,
    5000 of 40000).  Host sums the 8 per-core sum-exp partials per token
    (sharded logsumexp) - no collectives.
This cuts ScalarE exp work from 25.6M to ~18.4M elems/core (the hard
floor: exp runs only on ScalarE at 128 lanes * 1.2 GHz) and makes all
weights SBUF-resident.

The tail hidden states h = fp8((x8 @ 64*w1)/64) and the label-logit
dots are computed ON THE HOST (cheap: ~2.4 GFLOP numpy); the device
consumes the SAME fp8 h for the lse logits, so lse - label_logit
cancels fp8 noise exactly, and the device runs a single stream of
logit matmuls + exp with no phase-A, no gathers.  Total device input:
~6.5MB/core, all partition-major so every DMA is ~128-256 contiguous
descriptors (descriptor GENERATION was the startup bottleneck).

Per-tile reduction: ACT accum_out for t1 tile0; DVE tensor_reduce over
bf16 exp tiles for the rest.  Head/t0 tiles interleave between t1
tiles and real-shaped filler matmuls pad spare PSUM columns: the PE
HAM clock gate re-throttles the tensor engine to 1.2GHz when its
activity-window utilization drops, which would let ACT starve.

Numerics: fp8 DoubleRow matmuls with x64-scaled weights, undone for
free via exp(x/64); host finishes in float64 (log, masks, average).
"""

import numpy as np
import ml_dtypes

CUTOFF = [2000, 10000, 50000]
N_TOK = 4096
D = 1024
N_CORES = 8
TOK_PER_CORE = N_TOK // N_CORES          # 512
N_BLK = TOK_PER_CORE // 128              # 4 head token blocks
K0 = 8                                   # 1024/128
K1 = 2                                   # 256/128
N_HEAD = CUTOFF[0] + 2                   # 2002
V0 = CUTOFF[1] - CUTOFF[0]               # 8000
V1 = CUTOFF[2] - CUTOFF[1]               # 40000
V0S = V0 // N_CORES                      # 1000 per-core tail0 vocab slice
V1S = V1 // N_CORES                      # 5000 per-core tail1 vocab slice
WSCALE = 64.0
T1T = [2048, 1536, 1416]                 # t1 per-block vocab tile widths

BF16 = ml_dtypes.bfloat16
FP8 = ml_dtypes.float8_e4m3

_cache = {}


def _subs(width, step=512):
    out, o = [], 0
    while o < width:
        out.append((o, min(step, width - o)))
        o += min(step, width - o)
    return out


def _build_nc(b0, b1, use_bias):
    import concourse.bass as bass
    import concourse.bacc as bacc
    import concourse.mybir as mybir
    from concourse import tile

    t0c = b0 * 128
    t1c = b1 * 128
    nhh = (N_HEAD + 1023) // 1024        # head halves

    dt = mybir.dt
    nc = bacc.Bacc(None)

    EXP = mybir.ActivationFunctionType.Exp
    ADD = mybir.AluOpType.add
    DR = mybir.MatmulPerfMode.DoubleRow
    X = mybir.AxisListType.X
    PSUM = bass.MemorySpace.PSUM

    pre_p = nc.declare_dram_parameter("pre", [128, 16], dt.float8e4, isOutput=False)
    xh8_p = nc.declare_dram_parameter("xh8", [128, K0, TOK_PER_CORE], dt.float8e4, isOutput=False)
    hwt8_p = nc.declare_dram_parameter("hwt8", [128, nhh * 2, K0, 512], dt.float8e4, isOutput=False)
    if use_bias:
        hbias_p = nc.declare_dram_parameter("hbias", [1, N_HEAD], dt.bfloat16, isOutput=False)
    ht0_p = nc.declare_dram_parameter("ht0", [128, K0, t0c], dt.float8e4, isOutput=False)
    ht1_p = nc.declare_dram_parameter("ht1", [128, K1, t1c], dt.float8e4, isOutput=False)
    w2t0_p = nc.declare_dram_parameter("w2t0", [128, K0, V0S], dt.float8e4, isOutput=False)
    w2t1_p = nc.declare_dram_parameter("w2t1", [128, 3, K1, 2048], dt.float8e4, isOutput=False)

    ncols = (2 * N_BLK + 1) + b0 + 3 * b1
    out_s_p = nc.declare_dram_parameter("out_s", [128, ncols], dt.float32, isOutput=True)

    cols = []

    with tile.TileContext(nc) as tc:
        with (
            tc.tile_pool(name="res", bufs=1) as res,
            tc.tile_pool(name="es", bufs=12) as es,
        ):
            pre = res.tile([128, 16], dt.float8e4, tag="pre")
            xh8 = res.tile([128, K0, TOK_PER_CORE], dt.float8e4, tag="xh8")
            hwt8 = res.tile([128, nhh * 2, K0, 512], dt.float8e4, tag="hwt8")
            if use_bias:
                hbias = res.tile([1, N_HEAD], dt.bfloat16, tag="hbias")
            ht0_8 = res.tile([128, K0, t0c], dt.float8e4, tag="ht0_8")
            ht1_8 = res.tile([128, K1, t1c], dt.float8e4, tag="ht1_8")
            w2t0 = res.tile([128, K0, V0S], dt.float8e4, tag="w2t0")
            w2t1 = res.tile([128, 3, K1, 2048], dt.float8e4, tag="w2t1")
            sall = res.tile([128, ncols], dt.float32, tag="sall")
            ones = res.tile([128, 1], dt.bfloat16, tag="ones")
            ones1 = res.tile([1, 128], dt.bfloat16, tag="ones1")

            nc.gpsimd.memset(ones[:], 1.0)
            nc.gpsimd.memset(ones1[:], 1.0)

            with tc.tile_pool(name="pc", bufs=2, space=PSUM) as pcp:

                def exp_reduce(pc, w, mode, kind, b):
                    col = len(cols)
                    cols.append((kind, b))
                    if mode == "acc":
                        nc.scalar.activation(
                            pc[:, :w], pc[:, :w], EXP,
                            scale=1.0 / WSCALE, accum_out=sall[:, col:col + 1],
                        )
                    else:
                        et = es.tile([128, 2048], dt.bfloat16, tag="e")
                        nc.scalar.activation(et[:, :w], pc[:, :w], EXP,
                                             scale=1.0 / WSCALE)
                        nc.vector.tensor_reduce(sall[:, col:col + 1], et[:, :w],
                                                axis=X, op=ADD)

                def mm_group(pc, sl, btok, kk, lhs3, rhs3, bias=False, rbase=0,
                             bbase=0):
                    rsl = slice(rbase + sl.start, rbase + sl.stop)
                    for c in range(kk // 2):
                        nc.tensor.matmul(
                            pc[:, sl],
                            lhsT=lhs3[:, 2 * c:2 * c + 2, btok * 128:(btok + 1) * 128],
                            rhs=rhs3[:, 2 * c:2 * c + 2, rsl],
                            start=(c == 0),
                            stop=(c == kk // 2 - 1 and not bias),
                            perf_mode=DR,
                        )
                    if bias:
                        bsl = slice(bbase + sl.start, bbase + sl.stop)
                        nc.tensor.matmul(pc[:, sl], lhsT=ones1[:],
                                         rhs=hbias[0:1, bsl], start=False, stop=True)

                def pe_filler(pc, b, pofs=1536):
                    # real-shaped dummy matmul into unused PSUM columns of a
                    # narrow tile: PE-HAM keep-warm work; subtile deps keep
                    # the tile's ACT read independent of this write
                    bb = (b % b1) * 128
                    nc.tensor.matmul(
                        pc[:, pofs:pofs + 512],
                        lhsT=ht1_8[:, 0:2, bb:bb + 128],
                        rhs=w2t1[:, 0, 0:2, 0:512],
                        start=True, stop=True, perf_mode=DR,
                    )

                def emit_head(b, hf, split=False):
                    width = min(1024, N_HEAD - hf * 1024)
                    pc = pcp.tile([128, 2048], dt.float32, tag="pc")
                    for qi, (off, w) in enumerate(_subs(width)):
                        mm_group(pc, slice(off, off + w), b, K0, xh8,
                                 hwt8[:, hf * 2 + qi], bias=use_bias,
                                 rbase=-off, bbase=hf * 1024)
                        if split:
                            exp_reduce(pc[:, off:], w, "dve", "h", b)
                    if not split:
                        exp_reduce(pc, width, "dve", "h", b)

                def emit_t0(b):
                    pc = pcp.tile([128, 2048], dt.float32, tag="pc")
                    for off, w in _subs(V0S):
                        mm_group(pc, slice(off, off + w), b, K0, ht0_8, w2t0)
                    pe_filler(pc, b, 1024)
                    exp_reduce(pc, V0S, "dve", "t0", b)

                def emit_t1(b, j, j0mode="acc"):
                    width = T1T[j] if j < 2 else V1S - T1T[0] - T1T[1]
                    pc = pcp.tile([128, 2048], dt.float32, tag="pc")
                    for off, w in _subs(width):
                        mm_group(pc, slice(off, off + w), b, K1, ht1_8,
                                 w2t1[:, j])
                    if j >= 1:
                        pe_filler(pc, b + j, width)
                    mode = j0mode if j == 0 else (
                        "acc" if b >= b1 - 1 else "dve")
                    exp_reduce(pc, width, mode, "t1", b)

                # startup: tiny prewarm absorbs DMA spin-up; head inputs
                # first, then the t1 stream inputs; dummy matmuls warm the
                # PE HAM clock gate while the first DMAs land
                h1h = (t1c // 2) // 128 * 128
                nc.sync.dma_start(pre[:], pre_p[:])
                nc.sync.dma_start(xh8[:], xh8_p[:])
                nc.gpsimd.dma_start(hwt8[:, 0], hwt8_p[:, 0])
                if use_bias:
                    nc.sync.dma_start(hbias[:], hbias_p[:])
                nc.gpsimd.dma_start(hwt8[:, 1], hwt8_p[:, 1])
                nc.sync.dma_start(ht1_8[:, :, 0:h1h], ht1_p[:, :, 0:h1h])
                nc.sync.dma_start(w2t1[:, 0], w2t1_p[:, 0])
                emit_head(0, 0, split=True)
                nc.sync.dma_start(ht1_8[:, :, h1h:t1c], ht1_p[:, :, h1h:t1c])
                nc.sync.dma_start(w2t1[:, 1], w2t1_p[:, 1])
                emit_head(1, 0)
                nc.gpsimd.dma_start(hwt8[:, 2], hwt8_p[:, 2])
                nc.gpsimd.dma_start(hwt8[:, 3], hwt8_p[:, 3])
                nc.sync.dma_start(w2t1[:, 2], w2t1_p[:, 2])
                emit_head(2, 0)
                emit_head(1, 1)

                # spread head/t0 tiles (ACT-productive, PE-heavy) evenly
                # between the t1 blocks
                extras = [[] for _ in range(b1)]

                def put(bi, item):
                    extras[min(max(bi, 0), b1 - 1)].append(item)

                put(0, ("dma", "ht0"))
                put(1, ("dma", "w2t0"))
                hds = [(0, 1), (2, 1), (3, 0), (3, 1)]
                units = []
                for i in range(max(b0, len(hds))):
                    if i < len(hds):
                        units.append(("hd",) + hds[i])
                    if i < b0:
                        units.append(("t0", i))
                mix = list(range(2, b1))
                step = max(1.0, len(mix) / max(1, len(units)))
                for i, unit in enumerate(units):
                    put(mix[min(int(i * step), len(mix) - 1)], unit)

                def run_extra(e):
                    if e[0] == "hd":
                        emit_head(e[1], e[2])
                    elif e[0] == "t0":
                        emit_t0(e[1])
                    elif e[0] == "dma":
                        if e[1] == "ht0":
                            nc.sync.dma_start(ht0_8[:], ht0_p[:])
                        elif e[1] == "w2t0":
                            nc.sync.dma_start(w2t0[:], w2t0_p[:])

                flush_pts = sorted({max(0, b1 // 2), max(0, b1 - 1 - 1)})
                nflush = 0
                for b in range(b1):
                    u = extras[b]
                    heavy = any(e[0] in ("hd", "t0") for e in u)
                    j0mode = "dve" if (not heavy and b < b1 - 2) else "acc"
                    for j in range(3):
                        if j < len(u):
                            run_extra(u[j])
                        emit_t1(b, j, j0mode)
                    for e in u[3:]:
                        run_extra(e)
                    if b in flush_pts:
                        nc.sync.dma_start(out_s_p[:, nflush:len(cols)],
                                          sall[:, nflush:len(cols)])
                        nflush = len(cols)

            nc.scalar.dma_start(out_s_p[:, nflush:ncols], sall[:, nflush:ncols])

    nc.compile()
    return nc, cols


def _prep_inputs(w_in, target, head_w, head_b, tail0_w1, tail0_w2, tail1_w1, tail1_w2):
    f32 = np.float32
    w_in = np.asarray(w_in, f32)
    target = np.asarray(target).astype(np.int64)
    head_w = np.asarray(head_w, f32)
    head_b = np.asarray(head_b, f32)
    t0w1 = np.asarray(tail0_w1, f32)
    t0w2 = np.asarray(tail0_w2, f32)
    t1w1 = np.asarray(tail1_w1, f32)
    t1w2 = np.asarray(tail1_w2, f32)

    c0, c1, c2 = CUTOFF
    mask0 = (target >= c0) & (target < c1)
    mask1 = (target >= c1) & (target < c2)
    idx0 = np.where(mask0)[0]
    idx1 = np.where(mask1)[0]
    t0n, t1n = len(idx0), len(idx1)
    b0 = max(1, -(-t0n // 128))
    b1 = max(1, -(-t1n // 128))
    t0c, t1c = b0 * 128, b1 * 128
    nhh = (N_HEAD + 1023) // 1024
    first_t = np.where(mask0, c0, np.where(mask1, c0 + 1, target))
    use_bias = bool(np.any(head_b))

    def pmajor(a, k):
        # [k*128, F] -> [128, k, F] partition-major contiguous
        return np.ascontiguousarray(
            a.reshape(k, 128, a.shape[1]).transpose(1, 0, 2))

    x8_all = w_in.T.astype(FP8)                    # [1024, N_TOK]
    w1t0_8 = (t0w1.T * WSCALE).astype(FP8)         # [1024, 1024]
    w1t1_8 = (t1w1.T * WSCALE).astype(FP8)         # [1024, 256]
    hw_8 = (head_w.T * WSCALE).astype(FP8)         # [1024, 2002]
    w2t0_8 = (t0w2.T * WSCALE).astype(FP8)         # [1024, 8000]
    w2t1_8 = (t1w2.T * WSCALE).astype(FP8)         # [256, 40000]
    hbias = (head_b[None, :] * WSCALE).astype(BF16)

    # head weights, quarter-major [128, nhh*2, K0, 512]
    hw_pad = np.zeros((1024, nhh * 1024), FP8)
    hw_pad[:, :N_HEAD] = hw_8
    hwt8 = np.ascontiguousarray(
        hw_pad.reshape(K0, 128, nhh * 2, 512).transpose(1, 2, 0, 3))

    # ---- host-side tail hidden states + label-logit dots (the device
    # consumes the SAME fp8 h, so lse - dot cancels fp8 noise) ----
    f = np.float32
    h0 = ((x8_all[:, idx0].astype(f).T @ w1t0_8.astype(f)) / WSCALE).astype(FP8)
    h1 = ((x8_all[:, idx1].astype(f).T @ w1t1_8.astype(f)) / WSCALE).astype(FP8)
    ll0 = np.einsum("tf,ft->t", h0.astype(f),
                    w2t0_8[:, target[idx0] - c0].astype(f)) / WSCALE
    ll1 = np.einsum("tf,ft->t", h1.astype(f),
                    w2t1_8[:, target[idx1] - c1].astype(f)) / WSCALE
    llh = (np.einsum("ft,ft->t", x8_all.astype(f),
                     hw_8.astype(f)[:, first_t]) / WSCALE + head_b[first_t])

    def padT8(a, tcap):  # fp8 [T, F] -> fp8 [F, tcap]
        out = np.zeros((a.shape[1], tcap), FP8)
        out[:, :a.shape[0]] = a.T
        return out

    ht0 = pmajor(padT8(h0, t0c), K0)               # [128, K0, t0c]
    ht1 = pmajor(padT8(h1, t1c), K1)               # [128, K1, t1c]

    pre = np.zeros((128, 16), FP8)
    in_maps = []
    for c in range(N_CORES):
        sl = slice(c * TOK_PER_CORE, (c + 1) * TOK_PER_CORE)
        # per-core tail1 vocab slice, tile-major [128, 3, K1, 2048]
        w2t1s = np.zeros((256, 3, 2048), FP8)
        base = c * V1S
        o = 0
        for j, wdt in enumerate(T1T):
            w2t1s[:, j, :wdt] = w2t1_8[:, base + o:base + o + wdt]
            o += wdt
        w2t1m = np.ascontiguousarray(
            w2t1s.reshape(K1, 128, 3, 2048).transpose(1, 2, 0, 3))
        m = {
            "pre": pre,
            "xh8": pmajor(x8_all[:, sl], K0),
            "hwt8": hwt8,
            "ht0": ht0, "ht1": ht1,
            "w2t0": pmajor(w2t0_8[:, c * V0S:(c + 1) * V0S], K0),
            "w2t1": w2t1m,
        }
        if use_bias:
            m["hbias"] = hbias
        in_maps.append(m)
    meta = (b0, b1, t0n, t1n, use_bias, llh, ll0, ll1)
    return in_maps, meta


def _combine(results, cols, meta):
    b0, b1, t0n, t1n, use_bias, llh, ll0, ll1 = meta
    S0 = np.zeros((128, b0))
    S1 = np.zeros((128, b1))
    logSh = np.zeros(N_TOK)
    for c in range(N_CORES):
        S = results[c]["out_s"].astype(np.float64)
        Sh = np.zeros((128, N_BLK))
        for j, (k, b) in enumerate(cols):
            if k == "h":
                Sh[:, b] += S[:, j]
            elif k == "t0":
                S0[:, b] += S[:, j]
            else:
                S1[:, b] += S[:, j]
        # token (p, b) -> global index c*512 + b*128 + p
        logSh[c * TOK_PER_CORE:(c + 1) * TOK_PER_CORE] = np.log(Sh).T.reshape(-1)
    total = (logSh - llh).sum()
    total += (np.log(S0.T.reshape(-1)[:t0n]) - ll0).sum()
    total += (np.log(S1.T.reshape(-1)[:t1n]) - ll1).sum()
    return np.float32(total / N_TOK)


def _run(inputs, trace=False):
    from concourse.bass_utils import run_bass_kernel_spmd

    in_maps, meta = _prep_inputs(**inputs)
    key = (meta[0], meta[1], meta[4])
    if key not in _cache:
        _cache[key] = _build_nc(*key)
    nc, cols = _cache[key]
    res = run_bass_kernel_spmd(nc, in_maps, core_ids=list(range(N_CORES)), trace=trace)
    loss = _combine(res.results, cols, meta)
    return loss, res


def kernel(**inputs) -> np.ndarray:
    loss, _ = _run(inputs, trace=False)
    return loss
